# revision 1
# baseline (speedup 1.0000x reference)
"""KoLeo loss kernel for Trainium2 (8 NeuronCores, Bass/Tile).

reference semantics:
    x = student_output / max(||row||_2, 1e-8)        # [B, D] row-normalize
    dots = x @ x.T ; dots[i,i] = -1
    nn = argmax(dots, axis=1)
    d_i = || x_i - x_nn(i) + 1e-8 ||_2
    loss = mean(-log(d_i + 1e-8))

Device strategy (data-parallel over rows, 8 cores, identical NEFF):
  * Each core receives the full matrix cast to bf16 ("xbf") plus its own
    1024-row slice ("xlbf").
  * DMA-xbar-transposes put D on partitions: xT [128, 8, 8192] (raw) and
    xTl [128, 8, 1024] (raw local rows).
  * Per column tile j (512 cols): squares on GPSIMD + ones-matmul on PE
    give the replicated column-norm row n_j in PSUM [128, 512]; ACT Sqrt
    + DVE reciprocal -> rb_j = 1/n_j.
  * Main tiles: raw Gram v = x_local . x_all^T accumulated over 8 K-tiles
    in PSUM [128, 512]; the PSUM drain is fused with the column scaling:
    ct = v * rb_j  (= dots * n_i).  Per-tile top-8 (nc.vector.max) into a
    candidate buffer.  The scaled self-dot ct[i,i] = n_i (~32) dominates
    every other entry (~4), so the global 2nd-max of the row candidates is
    the nearest neighbor.  m_i = 2nd-max / n_i is the normalized NN dot.
  * d_i^2 = 2 - 2 m_i  (rows are unit-norm; the 1e-8 terms are far below
    f32 resolution for this data), output ln(d_i^2) per row.
Host: loss = -0.5 * sum(ln d^2) / B.
"""

import numpy as np
import ml_dtypes

import concourse.bacc as bacc
import concourse.bass as bass
import concourse.mybir as mybir
import concourse.tile as tile
from concourse import bass_utils

B, D, P = 8192, 1024, 128
NCORES = 8
LOCAL = B // NCORES  # 1024 rows per core
KT = D // P          # 8 contraction tiles
MT = LOCAL // P      # 8 local row tiles
NJ = 512             # moving free dim per matmul
JT = B // NJ         # 16 column tiles

F32 = mybir.dt.float32
BF16 = mybir.dt.bfloat16
AF = mybir.ActivationFunctionType


def emit_kernel(tc, x_ap, xl_ap, out_ap):
    nc = tc.nc
    with (
        tc.tile_pool(name="big", bufs=1) as big,
        tc.tile_pool(name="work", bufs=3) as work,
        tc.tile_pool(name="ps", bufs=4, space="PSUM") as pp,
        tc.tile_pool(name="ps2", bufs=2, space="PSUM") as pp2,
        tc.tile_pool(name="ps1", bufs=1, space="PSUM") as pp1,
    ):
        xT = big.tile([P, KT, B], BF16)
        xTl = big.tile([P, KT, LOCAL], BF16)
        cand = big.tile([P, MT, JT, 8], F32)
        ones = big.tile([P, P], BF16)
        rloc = big.tile([P, MT * 8], F32)
        d2t = big.tile([P, MT], F32)
        ltile = big.tile([P, MT], F32)

        nc.vector.memset(ones[:], 1.0)

        # warm the ACT function tables (Sqrt, Ln) before they gate anything
        warm = big.tile([P, 1], F32)
        nc.scalar.activation(warm[:], ones[:, :1], AF.Sqrt)
        nc.scalar.activation(warm[:], ones[:, :1], AF.Ln)
        nc.scalar.activation(warm[:], ones[:, :1], AF.Abs_reciprocal_sqrt)

        # --- loads: host ships x already transposed as [KT, 128, B], so
        # these are plain max-efficiency DMAs (128 partitions x contiguous
        # bytes), chunked in column ranges so the j-pipeline starts early,
        # and alternated across both HWDGE queues (SP / ACT).
        CH = 4
        CB = B // CH
        for k in range(KT):
            nc.sync.dma_start(out=xTl[:, k], in_=xl_ap[k])
        for c in range(CH):
            for k in range(KT):
                nc.sync.dma_start(
                    out=xT[:, k, c * CB : (c + 1) * CB],
                    in_=x_ap[k, :, c * CB : (c + 1) * CB],
                )

        # --- interleaved: column norms (one j ahead) + main Gram tiles ---
        rbs = {}

        def norm_stage(j):
            jb = slice(j * NJ, (j + 1) * NJ)
            # prologue stages square on DVE (idle then); steady state on GPSIMD
            sqeng = nc.vector if (j < 2 or j % 2 == 1) else nc.gpsimd
            # eager square-and-accumulate over k so the partition reduction
            # needs a single ones-matmul
            acc = work.tile([P, NJ], BF16, tag="sqa")
            sqb = work.tile([P, NJ], BF16, tag="sqb")
            sqeng.tensor_mul(acc[:], xT[:, 0, jb], xT[:, 0, jb])
            for k in range(1, KT):
                sqeng.tensor_mul(sqb[:], xT[:, k, jb], xT[:, k, jb])
                sqeng.tensor_add(acc[:], acc[:], sqb[:])
            psum_s = pp2.tile([P, NJ], F32, tag="ps_s")
            nc.tensor.matmul(psum_s[:], ones[:], acc[:], start=True, stop=True)
            rb = work.tile([P, NJ], F32, tag="rb")
            nc.scalar.activation(rb[:], psum_s[:], AF.Abs_reciprocal_sqrt)
            rbs[j] = rb

        norm_stage(0)
        norm_stage(1)

        # --- local row norms (eager square-accumulate on DVE; emitted after
        # the prologue norm stages so they don't gate the first drains) ---
        lacc = big.tile([P, LOCAL], BF16)
        lsqb = big.tile([P, LOCAL], BF16)
        nc.vector.tensor_mul(lacc[:], xTl[:, 0], xTl[:, 0])
        for k in range(1, KT):
            nc.vector.tensor_mul(lsqb[:], xTl[:, k], xTl[:, k])
            nc.vector.tensor_add(lacc[:], lacc[:], lsqb[:])
        for mt in range(MT):
            psum_l = pp1.tile([P, 8], F32, tag="ps_l")
            nc.tensor.matmul(
                psum_l[:],
                lacc[:, mt * P : (mt + 1) * P],
                ones[:, :8],
                start=True,
                stop=True,
            )
            nc.scalar.activation(
                rloc[:, mt * 8 : (mt + 1) * 8], psum_l[:], AF.Abs_reciprocal_sqrt
            )

        for j in range(JT):
            jb = slice(j * NJ, (j + 1) * NJ)
            rb = rbs.pop(j)

            # main: v = x_local_raw @ x_raw.T, drain fused with * (1/n_j)
            for mt in range(MT):
                psum_u = pp.tile([P, NJ], F32, tag="ps_u")
                for k in range(KT):
                    nc.tensor.matmul(
                        psum_u[:],
                        xTl[:, k, mt * P : (mt + 1) * P],
                        xT[:, k, jb],
                        start=(k == 0),
                        stop=(k == KT - 1),
                    )
                ct = work.tile([P, NJ], F32, tag="ct")
                nc.vector.tensor_mul(ct[:], psum_u[:], rb[:])
                nc.vector.max(out=cand[:, mt, j], in_=ct[:])
            if j + 2 < JT:
                norm_stage(j + 2)

        # --- finalize: 2nd max -> m_i -> ln(d^2) ---
        for mt in range(MT):
            c8 = work.tile([P, 8], F32, tag="c8")
            nc.vector.max(out=c8[:], in_=cand[:, mt])
            mi = work.tile([P, 1], F32, tag="mi")
            nc.vector.tensor_mul(mi[:], c8[:, 1:2], rloc[:, mt * 8 : mt * 8 + 1])
            nc.vector.tensor_scalar(
                d2t[:, mt : mt + 1],
                mi[:],
                -2.0,
                2.0,
                op0=mybir.AluOpType.mult,
                op1=mybir.AluOpType.add,
            )
        nc.scalar.activation(ltile[:], d2t[:], AF.Ln)
        nc.sync.dma_start(out=out_ap, in_=ltile[:])


def build_bass():
    nc = bacc.Bacc(
        "TRN2",
        target_bir_lowering=False,
        debug=False,
        enable_asserts=True,
        num_devices=NCORES,
    )
    x_t = nc.dram_tensor("xbf", [KT, P, B], BF16, kind="ExternalInput").ap()
    xl_t = nc.dram_tensor("xlbf", [KT, P, LOCAL], BF16, kind="ExternalInput").ap()
    out_t = nc.dram_tensor("lnd2", [P, MT], F32, kind="ExternalOutput").ap()
    with tile.TileContext(nc) as tc:
        emit_kernel(tc, x_t, xl_t, out_t)
    nc.compile()
    return nc


def make_in_maps(x: np.ndarray):
    xbf = x.astype(ml_dtypes.bfloat16)
    # [KT, P, B]: element [k, p, r] = x[r, k*128 + p]  (transposed layout)
    xt = np.ascontiguousarray(xbf.reshape(B, KT, P).transpose(1, 2, 0))
    return [
        {
            "xbf": xt,
            "xlbf": np.ascontiguousarray(xt[:, :, c * LOCAL : (c + 1) * LOCAL]),
        }
        for c in range(NCORES)
    ]


def reduce_outputs(results):
    total = 0.0
    for r in results:
        total += float(r["lnd2"].astype(np.float64).sum())
    return np.array(-0.5 * total / B, dtype=np.float32)


_LAST_RESULTS = None  # BassKernelResults of the most recent run (for test.py)


def run(x: np.ndarray, trace: bool = False):
    global _LAST_RESULTS
    nc = build_bass()
    res = bass_utils.run_bass_kernel_spmd(
        nc,
        make_in_maps(x),
        core_ids=list(range(NCORES)),
        trace=trace,
        trace_cores=list(range(NCORES)) if trace else None,
    )
    _LAST_RESULTS = res
    return reduce_outputs(res.results)


def kernel(**inputs) -> np.ndarray:
    x = np.asarray(inputs["student_output"], dtype=np.float32)
    assert x.shape == (B, D), x.shape
    return run(x, trace=False)


if __name__ == "__main__":
    rng = np.random.default_rng(0)
    x = rng.standard_normal((B, D), dtype=np.float32)
    print(kernel(student_output=x))



# revision 5
# speedup vs baseline: 2.0803x; 2.0803x over previous
"""KoLeo loss kernel for Trainium2 (8 NeuronCores, Bass/Tile), fp8 edition.

reference semantics:
    x = student_output / max(||row||_2, 1e-8)        # [B, D] row-normalize
    dots = x @ x.T ; dots[i,i] = -1
    nn = argmax(dots, axis=1)
    d_i = || x_i - x_nn(i) + 1e-8 ||_2
    loss = mean(-log(d_i + 1e-8))

Device strategy (data-parallel over rows, 8 cores, identical NEFF):
  * Host pre-normalizes rows in fp32, scales by S=128, quantizes to
    fp8 e4m3 and ships the transposed layout [KT=8, 128, B] (element
    [k, p, r] = q[r, k*128+p]).  Max |q| <= S < 240 so e4m3 is safe;
    numpy-validated end-to-end rel err ~1.5e-4.
  * Each core gets the full matrix ("xq", 8 MB) plus its own 1024-row
    slice ("xlq", 1 MB).
  * Gram tiles run as fp8 DoubleRow matmuls: each MM consumes TWO
    128-deep k-tiles ([128, 2, free] APs) at 0.5 cycles/row — 2x bf16
    PE throughput.  Rows are unit-norm pre-quantization, so no column
    scaling is needed: the diagonal entry is ~S^2 (>= 5x everything
    else), so the global 2nd-max of a row is S^2 * the NN cosine.
  * Loop: jh-stripes of 2048 columns outer (compute starts after 2 MB
    of DMA), mt (128-row chunk) inner; per (jh, mt) group 4 psum tiles
    [128, 512] accumulate 4 DoubleRow MMs each; DVE max8 drains PSUM
    directly into a candidate buffer.  4 psum tags x 2 bufs = all 8
    PSUM banks ping-pong; dummy warm-up matmuls run during the first
    DMA so the PE/HAM is at full clock when real tiles land.
  * Finalize per mt: global max8 over the 16*8 candidates, take the
    2nd value m~, d^2 = 2 - (2/S^2) m~, emit ln(d^2) per row.
Host: loss = -0.5 * sum(ln d^2) / B.
"""

import numpy as np
import ml_dtypes

import concourse.bacc as bacc
import concourse.bass as bass
import concourse.mybir as mybir
import concourse.tile as tile
from concourse import bass_utils

B, D, P = 8192, 1024, 128
NCORES = 8
LOCAL = B // NCORES  # 1024 rows per core
KT = D // P          # 8 contraction tiles
MT = LOCAL // P      # 8 local row tiles
NJ = 512             # moving free dim per matmul
JT = B // NJ         # 16 column tiles
JH = 4               # j-tiles per stripe
NSTRIPE = JT // JH   # 4 stripes of 2048 columns
SCALE = 128.0        # fp8 pre-scale; diag ~ S^2

F32 = mybir.dt.float32
FP8 = mybir.dt.float8e4
AF = mybir.ActivationFunctionType
DR = mybir.MatmulPerfMode.DoubleRow


def emit_kernel(tc, x_ap, xl_ap, out_ap):
    nc = tc.nc
    with (
        tc.tile_pool(name="big", bufs=1) as big,
        tc.tile_pool(name="work", bufs=3) as work,
        tc.tile_pool(name="ps", bufs=2, space="PSUM") as pp,
    ):
        xT = big.tile([P, KT, B], FP8)
        xTl = big.tile([P, KT, LOCAL], FP8)
        cand = big.tile([P, MT, JT, 8], F32)
        d2t = big.tile([P, MT], F32)
        ltile = big.tile([P, MT], F32)
        warm = big.tile([P, NJ], FP8)

        nc.vector.memset(warm[:], 1.0)

        # warm the ACT Ln table early so it doesn't gate the finalize
        wact = big.tile([P, 1], F32)
        wsrc = big.tile([P, 1], F32)
        nc.vector.memset(wsrc[:], 1.0)
        nc.scalar.activation(wact[:], wsrc[:], AF.Ln)

        # --- loads: host ships fp8 already transposed as [KT, 128, B];
        # stripe-chunked so the first Gram stripe starts after ~2 MB.
        for k in range(KT):
            nc.sync.dma_start(out=xTl[:, k], in_=xl_ap[k])
        for s in range(NSTRIPE):
            cb = slice(s * JH * NJ, (s + 1) * JH * NJ)
            for k in range(KT):
                nc.sync.dma_start(out=xT[:, k, cb], in_=x_ap[k, :, cb])

        # --- PE/HAM pre-warm: dummy matmuls on the memset tile while the
        # first stripe DMA is in flight (borrows one main-pool psum buffer;
        # finishes long before its rotation comes around again).
        wps = pp.tile([P, NJ], F32, tag="ps_u0", name="wps")
        for _ in range(12):
            nc.tensor.matmul(wps[:], warm[:, :P], warm[:], start=True, stop=True)

        # --- Gram: fp8 DoubleRow, 2 k-tiles per MM ------------------------
        for s in range(NSTRIPE):
            for mt in range(MT):
                ms = slice(mt * P, (mt + 1) * P)
                pss = [
                    pp.tile([P, NJ], F32, tag=f"ps_u{jj}", name=f"ps_u{jj}")
                    for jj in range(JH)
                ]
                for kk in range(KT // 2):
                    ks = slice(2 * kk, 2 * kk + 2)
                    for jj in range(JH):
                        j = s * JH + jj
                        nc.tensor.matmul(
                            pss[jj][:],
                            xTl[:, ks, ms],
                            xT[:, ks, j * NJ : (j + 1) * NJ],
                            start=(kk == 0),
                            stop=(kk == KT // 2 - 1),
                            perf_mode=DR,
                        )
                for jj in range(JH):
                    nc.vector.max(out=cand[:, mt, s * JH + jj], in_=pss[jj][:])

        # --- finalize: global 2nd max -> m~ -> ln(d^2) --------------------
        inv = 2.0 / (SCALE * SCALE)
        for mt in range(MT):
            c8 = work.tile([P, 8], F32, tag="c8")
            nc.vector.max(out=c8[:], in_=cand[:, mt])
            nc.vector.tensor_scalar(
                d2t[:, mt : mt + 1],
                c8[:, 1:2],
                -inv,
                2.0,
                op0=mybir.AluOpType.mult,
                op1=mybir.AluOpType.add,
            )
        nc.scalar.activation(ltile[:], d2t[:], AF.Ln)
        nc.sync.dma_start(out=out_ap, in_=ltile[:])


def build_bass():
    nc = bacc.Bacc(
        "TRN2",
        target_bir_lowering=False,
        debug=False,
        enable_asserts=True,
        num_devices=NCORES,
    )
    x_t = nc.dram_tensor("xq", [KT, P, B], FP8, kind="ExternalInput").ap()
    xl_t = nc.dram_tensor("xlq", [KT, P, LOCAL], FP8, kind="ExternalInput").ap()
    out_t = nc.dram_tensor("lnd2", [P, MT], F32, kind="ExternalOutput").ap()
    with tile.TileContext(nc) as tc:
        emit_kernel(tc, x_t, xl_t, out_t)
    nc.compile()
    return nc


def make_in_maps(x: np.ndarray):
    norm = np.linalg.norm(x, axis=1, keepdims=True)
    xn = x / np.maximum(norm, 1e-8)
    q = (SCALE * xn).astype(ml_dtypes.float8_e4m3)
    # [KT, P, B]: element [k, p, r] = q[r, k*128 + p]  (transposed layout)
    xt = np.ascontiguousarray(q.reshape(B, KT, P).transpose(1, 2, 0))
    return [
        {
            "xq": xt,
            "xlq": np.ascontiguousarray(xt[:, :, c * LOCAL : (c + 1) * LOCAL]),
        }
        for c in range(NCORES)
    ]


def reduce_outputs(results):
    total = 0.0
    for r in results:
        total += float(r["lnd2"].astype(np.float64).sum())
    return np.array(-0.5 * total / B, dtype=np.float32)


_LAST_RESULTS = None  # BassKernelResults of the most recent run (for test.py)


def run(x: np.ndarray, trace: bool = False):
    global _LAST_RESULTS
    nc = build_bass()
    res = bass_utils.run_bass_kernel_spmd(
        nc,
        make_in_maps(x),
        core_ids=list(range(NCORES)),
        trace=trace,
        trace_cores=list(range(NCORES)) if trace else None,
    )
    _LAST_RESULTS = res
    return reduce_outputs(res.results)


def kernel(**inputs) -> np.ndarray:
    x = np.asarray(inputs["student_output"], dtype=np.float32)
    assert x.shape == (B, D), x.shape
    return run(x, trace=False)


if __name__ == "__main__":
    rng = np.random.default_rng(0)
    x = rng.standard_normal((B, D), dtype=np.float32)
    print(kernel(student_output=x))
